# revision 65
# baseline (speedup 1.0000x reference)
"""AttentionPooling (segment softmax-pool) Trainium2 kernel.

Math (per reference):
    h      = gelu(x @ W1 + b1)            # [N, H]
    s      = h @ W2 + b2                  # [N]
    w      = softmax_per_segment(s)       # segments from sorted `batch`
    pooled = segment_sum(w[:, None] * x)  # [B, D]

Strategy (8 NeuronCores, data-parallel over N):
  - Shard rows across 8 cores. Each core streams its rows once in bf16
    natural layout (pooling operand) and once in fp8-e4m3 transposed
    layout (score-MLP operand; scores tolerate fp8 -- verified 5e-3 rel
    err vs 2e-2 budget). That is 96 MB/core vs 128 MB for bf16 twice.
  - Scores via the tiny MLP on the tensor engine (fp8 moving x, bf16 W1
    stationary, f32 psum), gelu batched 3 macros per ACT instruction.
  - e = exp(s + b2) batched over E=5 groups to amortize the ACT
    gelu<->exp table swap (1.3 us each).
  - Pooling: one-hot-times-e matrix A[row, seg-in-window] built with
    iota/is_equal on the vector engine (window = [b_lo_m, b_lo_m + W)),
    windowed pooled partials P_m[W, D] = sum_rows e_i * x_i via matmul
    (A moving, natural-x stationary), f32 PSUM accumulation.
  - Device ships per-macro windows P_m (bf16) and per-row e back to HBM.
  - Host scatter-adds the windows at their (host-known) b_lo_m offsets,
    computes denominators from e, combines the 8 cores, and divides.
    Softmax max-subtraction is skipped: scores are O(1) for this model,
    and softmax is shift-invariant, so exp() cannot overflow.
"""

import sys

import numpy as np

sys.path.insert(0, "/opt/trn_rl_repo")

import ml_dtypes

N_CORES = 8
D = 128  # feature dim
H = 128  # hidden dim
NSEG = 1024
PAD_SEG = NSEG  # extra segment id for padding rows
CHUNK = 128  # rows per PE contraction
CH = 4  # chunks per macro
MACRO = CHUNK * CH  # 512 rows
KST = 14  # macros per group (DMA batch)
TRI = 2  # macros per gelu batch
LAG = 2  # groups between a group's scores pass and its pooling pass

_prog_cache: dict = {}


def _build_program(NM: int, W: int, act_name: str = "Gelu"):
    """Emit + compile the per-core Tile program. NM macros per core (multiple
    of KST), segment window W."""
    from contextlib import ExitStack

    import concourse.tile as tile
    from concourse import bacc, mybir

    bf16 = mybir.dt.bfloat16
    f8 = mybir.dt.float8e4
    f32 = mybir.dt.float32
    AF = mybir.ActivationFunctionType
    ALU = mybir.AluOpType
    DR = mybir.MatmulPerfMode.DoubleRow

    assert NM % KST == 0 and KST % TRI == 0
    NG = NM // KST
    Nc = NM * MACRO
    NTRI = KST // TRI

    nc = bacc.Bacc("TRN2", target_bir_lowering=False, debug=False, num_devices=N_CORES)

    xap = nc.dram_tensor("xap", [CHUNK, NM, CH, D], bf16, kind="ExternalInput")
    xhit = nc.dram_tensor("xhit", [D, Nc], f8, kind="ExternalInput")
    brel = nc.dram_tensor("brel", [128, NM, CH], bf16, kind="ExternalInput")
    w1 = nc.dram_tensor("w1", [D, H], bf16, kind="ExternalInput")
    w2 = nc.dram_tensor("w2", [H, 1], bf16, kind="ExternalInput")
    b1 = nc.dram_tensor("b1", [H, 1], f32, kind="ExternalInput")
    b2 = nc.dram_tensor("b2", [128, 1], f32, kind="ExternalInput")
    iota = nc.dram_tensor("iota", [128, W], bf16, kind="ExternalInput")
    pool_out = nc.dram_tensor("pool_out", [D, NM, W], bf16, kind="ExternalOutput")
    e_out = nc.dram_tensor("e_out", [128, NM, CH], bf16, kind="ExternalOutput")

    # DRAM views (both host-prepped layouts are contiguous per partition)
    xa_view = xap.ap().rearrange("p (g k) j d -> g p k j d", k=KST)
    xt_view = xhit.ap().rearrange("d (g k r) -> g d k r", k=KST, r=MACRO)

    with tile.TileContext(nc) as tc, ExitStack() as ctx:
        pool = lambda name, bufs, **kw: ctx.enter_context(
            tc.tile_pool(name=name, bufs=bufs, **kw)
        )
        p_const = pool("const", 1)
        p_xa = pool("xa", 6)
        p_xt = pool("xt", 4)
        p_bt = pool("bt", 6)
        p_hg = pool("hg", 3)
        p_msk = pool("msk", 3)
        p_a = pool("amat", 24)
        p_es = pool("estage", 8)
        p_ps = pool("pstage", 4)
        p_hp = pool("hpsum", 2, space="PSUM")  # 2 banks each
        p_sc = pool("scpsum", 2, space="PSUM")  # 1 bank each
        p_pp = pool("ppsum", 2, space="PSUM")  # 1 bank each
        p_tsc = pool("tanh", 3)
        p_ex = pool("expchain", 6)

        w1_sb = p_const.tile([D, H], bf16)
        nc.sync.dma_start(w1_sb[:], w1.ap())
        w2_sb = p_const.tile([H, 1], bf16)
        nc.sync.dma_start(w2_sb[:], w2.ap())
        b1_sb = p_const.tile([H, 1], f32)
        nc.sync.dma_start(b1_sb[:], b1.ap())
        b2_sb = p_const.tile([128, 1], f32)
        nc.sync.dma_start(b2_sb[:], b2.ap())
        iota_sb = p_const.tile([128, W], bf16)
        nc.sync.dma_start(iota_sb[:], iota.ap())

        e_t: dict = {}
        xt_t: dict = {}
        xa_t: dict = {}
        bt_t: dict = {}
        amat_t: dict = {}

        def issue_xt(s):
            if 0 <= s < NG and s not in xt_t:
                t = p_xt.tile([128, KST, MACRO], f8)
                if s < 2:
                    # split the startup transfers so the first mm1s begin
                    # after ~1/3 of the tile instead of all of it
                    for lo, hi in ((0, 6), (6, 10), (10, KST)):
                        nc.gpsimd.dma_start(t[:, lo:hi, :], xt_view[s][:, lo:hi, :])
                else:
                    nc.gpsimd.dma_start(t[:], xt_view[s])
                xt_t[s] = t

        def issue_xa(p):
            if 0 <= p < NG and p not in xa_t:
                t = p_xa.tile([128, KST, CH, CHUNK], bf16)
                nc.gpsimd.dma_start(t[:], xa_view[p])
                xa_t[p] = t
                b = p_bt.tile([128, KST, CH], bf16)
                nc.gpsimd.dma_start(b[:], brel.ap()[:, p * KST : (p + 1) * KST, :])
                bt_t[p] = b

        issue_xt(0)
        issue_xt(1)
        for q in range(4):
            issue_xa(q)

        # Flat software pipeline over macro-triplet slots; score slot n
        # covers (g, it) = divmod(n, NTRI). Per slot, in emission order:
        #   - mm1 triplet for slot n+1 (one slot of PE lookahead, so the
        #     next gelu's inputs are never queued behind this slot's pool
        #     and mm2 matmuls on the in-order PE); group-entry bookkeeping
        #     (dma issues, A-matrix builds for group g-2) rides with it
        #   - gelu for slot n, then the deferred tanh/exp chain of the
        #     previous group (rides the saturated ACT queue, no table swap)
        #   - pool matmuls + psum evacuation for slot n - POOL_SHIFT
        #   - mm2s for slot n-1 (one-slot delay so they never wait on gelu)
        # Group g's scores complete at slot 5g+5 and are staged to SBUF;
        # the tanh fires after the next gelu, the e-chain runs on the DVE,
        # and g's A matrices are built at the entry of group g+2 -- a full
        # group before the pool matmuls of group g (entry g+3) want them.
        POOL_SHIFT = LAG * NTRI
        NJ = NG * NTRI

        def emit_amat(q, kk):
            qbt = bt_t[q]
            msk = p_msk.tile([128, TRI, CH, W], bf16)
            nc.vector.tensor_tensor(
                out=msk[:],
                in0=iota_sb[:]
                .unsqueeze(1)
                .unsqueeze(2)
                .broadcast_to([128, TRI, CH, W]),
                in1=qbt[:, kk : kk + TRI, :]
                .unsqueeze(3)
                .broadcast_to([128, TRI, CH, W]),
                op=ALU.is_equal,
            )
            amat = p_a.tile([128, TRI, CH, W], bf16)
            nc.vector.tensor_tensor(
                out=amat[:],
                in0=msk[:],
                in1=e_t[q][:, kk : kk + TRI, :]
                .unsqueeze(3)
                .broadcast_to([128, TRI, CH, W]),
                op=ALU.mult,
            )
            return amat

        def group_entry(g):
            issue_xt(g + 2)
            issue_xa(g)
            q = g - 2
            if 0 <= q < NG:
                amat_t[q] = [emit_amat(q, jt * TRI) for jt in range(NTRI)]
                bt_t.pop(q)
                e_t.pop(q)

        def emit_mm1(n):
            if not 0 <= n < NJ:
                if n < NJ + POOL_SHIFT and n % NTRI == 0:
                    group_entry(n // NTRI)
                return None
            g, it = divmod(n, NTRI)
            kk = it * TRI
            if it == 0:
                group_entry(g)
            hp = p_hp.tile([128, TRI, MACRO], f32, space="PSUM")
            for i in range(TRI):
                nc.tensor.matmul(
                    hp[:, i, :], lhsT=w1_sb[:], rhs=xt_t[g][:, kk + i, :],
                    start=True, stop=True,
                )
            if it == NTRI - 1:
                xt_t.pop(g, None)
            return hp

        def emit_e(g, scb):
            # e = exp(s + b2) via the tanh table (same ACT table set as
            # gelu -- no table swap): e = (1+th)/(1-th), th = tanh((s+b2)/2),
            # read straight from the score psum. The divide runs on the DVE
            # (native accurate reciprocal).
            etile = p_es.tile([128, KST, CH], bf16)
            tsc = p_tsc.tile([128, KST, CH], f32)
            nc.scalar.activation(
                tsc[:].rearrange("p k j -> p (k j)"),
                scb[:].rearrange("p k j -> p (k j)"),
                AF.Tanh,
                bias=b2_sb[:],  # holds b2/2 (host-prepped)
                scale=0.5,
            )
            den = p_ex.tile([128, KST, CH], f32)
            nc.vector.tensor_scalar(den[:], tsc[:], -1.0, 1.0, ALU.mult, ALU.add)
            rec = p_ex.tile([128, KST, CH], f32)
            nc.vector.reciprocal(rec[:], den[:])
            num = p_ex.tile([128, KST, CH], f32)
            nc.vector.tensor_scalar_add(num[:], tsc[:], 1.0)
            nc.vector.tensor_tensor(out=etile[:], in0=num[:], in1=rec[:], op=ALU.mult)
            e_t[g] = etile
            nc.sync.dma_start(e_out.ap()[:, g * KST : (g + 1) * KST, :], etile[:])

        e_pend = None
        sc_cur = None
        hg_prev = None
        hp_next = None
        pool_st = None  # (pstage, pxa, amats, pp_t) of the active pool group

        for n in range(NJ + POOL_SHIFT):
            if n == 0:
                hp_next = emit_mm1(0)
            hp_cur = hp_next
            hp_next = emit_mm1(n + 1)

            if n >= NJ and e_pend is not None:
                emit_e(*e_pend)
                e_pend = None

            # gelu for slot n, then the pending tanh/exp chain
            if n < NJ:
                g, it = divmod(n, NTRI)
                hg = p_hg.tile([128, TRI, MACRO], bf16)
                nc.scalar.activation(
                    hg[:].rearrange("p i r -> p (i r)"),
                    hp_cur[:].rearrange("p i r -> p (i r)"),
                    getattr(AF, act_name),
                    bias=b1_sb[:],
                    scale=1.0,
                )
                if it >= 2 and e_pend is not None:
                    # two gelu slots after the pend was set: by now the mm2
                    # flush and scb copy it depends on have certainly run,
                    # so the tanh never stalls the in-order ACT queue
                    emit_e(*e_pend)
                    e_pend = None

            # pool pass for slot n - POOL_SHIFT
            m = n - POOL_SHIFT
            if m >= 0:
                gp, itp = divmod(m, NTRI)
                if itp == 0:
                    pstage = p_ps.tile([D, KST, W], bf16)
                    pool_st = (pstage, xa_t.pop(gp), amat_t.pop(gp))
                pstage, pxa, amats = pool_st
                kkp = itp * TRI
                # fresh one-bank psum per slot: PSUM accumulation-group
                # starts are zero-region (whole bank) scoped, so slots must
                # rotate across banks to overlap with the evacuating casts
                pp_t = p_pp.tile([D, TRI, W], f32, space="PSUM")
                for i in range(TRI):
                    for j in range(CH):
                        nc.tensor.matmul(
                            pp_t[:, i, :],
                            lhsT=pxa[:, kkp + i, j, :],
                            rhs=amats[itp][:, i, j, :],
                            start=(j == 0),
                            stop=(j == CH - 1),
                        )
                nc.vector.tensor_copy(pstage[:, kkp : kkp + TRI, :], pp_t[:])
                if itp == NTRI - 1:
                    # outputs ride the sync HWDGE queue so their (trailing)
                    # sem waits never block the gpsimd input-prefetch queue
                    nc.sync.dma_start(
                        pool_out.ap()[:, gp * KST : (gp + 1) * KST, :], pstage[:]
                    )

            # mm2s for slot n-1 (into the score psum of that slot's group)
            if 1 <= n <= NJ:
                gm, itm = divmod(n - 1, NTRI)
                if itm == 0:
                    sc_cur = p_sc.tile([128, KST, CH], f32, space="PSUM")
                kkm = itm * TRI
                for ii in range(TRI):
                    for j in range(CH):
                        nc.tensor.matmul(
                            sc_cur[:, kkm + ii, j : j + 1],
                            lhsT=hg_prev[:, ii, j * CHUNK : (j + 1) * CHUNK],
                            rhs=w2_sb[:],
                            start=True,
                            stop=True,
                        )
                if itm == NTRI - 1:
                    # sc is double-buffered, so the finished scores stay in
                    # psum until the tanh reads them two gelu slots later
                    e_pend = (gm, sc_cur)

            if n < NJ:
                hg_prev = hg

    nc.compile()
    return nc


def _prep_inputs(x, batch, W1, b1, W2, b2):
    """Host-side shard + preprocess. Returns (in_maps, meta)."""
    bf = ml_dtypes.bfloat16
    f8 = ml_dtypes.float8_e4m3fn
    x = np.asarray(x)
    batch = np.asarray(batch)
    N = x.shape[0]

    NM = -(-N // (N_CORES * MACRO))  # macros per core
    NM = -(-NM // KST) * KST  # round up to full groups
    NP = N_CORES * NM * MACRO
    Nc = NM * MACRO

    xhi = np.zeros((NP, D), dtype=bf)
    xhi[:N] = x.astype(bf)
    bpad = np.full(NP, PAD_SEG, dtype=np.int64)
    bpad[:N] = batch

    bv = bpad.reshape(N_CORES, NM, MACRO)
    # window start per macro; pad id is the largest so min() tracks real rows
    blo = bv.min(axis=2)  # [8, NM]
    # window width from real rows only
    real = bv != PAD_SEG
    breal_max = np.where(real, bv, -1).max(axis=2)  # -1 if all pad
    span = np.maximum(breal_max - blo + 1, 1)
    W = int(max(8, span.max()))
    assert W <= 128, f"segment window {W} too wide"

    brel = (bv - blo[:, :, None]).astype(bf)  # [8, NM, 512]
    # device layout: brel_dev[c, p, m, j] = brel[c, m, j*128 + p]
    brel_dev = np.ascontiguousarray(
        brel.reshape(N_CORES, NM, CH, CHUNK).transpose(0, 3, 1, 2)
    )

    iota_arr = np.ascontiguousarray(
        np.broadcast_to(np.arange(W, dtype=np.float32).astype(bf), (128, W))
    )
    w1c = np.ascontiguousarray(np.asarray(W1).astype(bf))
    w2c = np.ascontiguousarray(np.asarray(W2).astype(bf))
    b1c = np.ascontiguousarray(np.asarray(b1, dtype=np.float32).reshape(H, 1))
    # the device uses b2 only as the tanh bias: tanh((s + b2)/2) needs b2/2
    b2c = np.full(
        (128, 1), np.asarray(b2, dtype=np.float32).ravel()[0] / 2.0, np.float32
    )

    in_maps = []
    for c in range(N_CORES):
        xc = xhi[c * Nc : (c + 1) * Nc]
        in_maps.append(
            {
                # xap[p, m, j, :] = x[m*512 + j*128 + p, :]
                "xap": np.ascontiguousarray(
                    xc.reshape(NM, CH, CHUNK, D).transpose(2, 0, 1, 3)
                ),
                "xhit": np.ascontiguousarray(xc.T.astype(f8)),
                "brel": brel_dev[c],
                "w1": w1c,
                "w2": w2c,
                "b1": b1c,
                "b2": b2c,
                "iota": iota_arr,
            }
        )
    meta = {"NM": NM, "W": W, "Nc": Nc, "NP": NP, "N": N, "blo": blo, "bpad": bpad}
    return in_maps, meta


def _combine(results, meta):
    """Host unshard: scatter-add macro windows, divide by segment denominators."""
    NM, W, Nc = meta["NM"], meta["W"], meta["Nc"]
    blo, bpad = meta["blo"], meta["bpad"]

    seg_acc = np.zeros((NSEG + 1, D), dtype=np.float64)
    e_all = np.empty(N_CORES * Nc, dtype=np.float32)
    wofs = np.arange(W)
    for c in range(N_CORES):
        po = np.asarray(results[c]["pool_out"], dtype=np.float64)  # [D, NM, W]
        seg_idx = (blo[c][:, None] + wofs[None, :]).ravel()  # [NM*W]
        valid = seg_idx <= NSEG
        contrib = po.transpose(1, 2, 0).reshape(-1, D)  # [NM*W, D]
        np.add.at(seg_acc, seg_idx[valid], contrib[valid])
        # e_dev[p, m, j] -> row m*512 + j*128 + p
        e_dev = np.asarray(results[c]["e_out"]).astype(np.float32)  # [128, NM, CH]
        e_all[c * Nc : (c + 1) * Nc] = e_dev.transpose(1, 2, 0).reshape(Nc)

    denom = np.bincount(bpad, weights=e_all.astype(np.float64), minlength=NSEG + 1)
    denom = denom[:NSEG]
    out = seg_acc[:NSEG]
    safe = denom != 0
    pooled = np.zeros((NSEG, D), dtype=np.float32)
    pooled[safe] = (out[safe] / denom[safe, None]).astype(np.float32)
    return pooled


def _run(inputs: dict, trace: bool = False):
    from concourse.bass_utils import run_bass_kernel_spmd

    in_maps, meta = _prep_inputs(
        inputs["x"], inputs["batch"], inputs["W1"], inputs["b1"], inputs["W2"],
        inputs["b2"],
    )
    key = (meta["NM"], meta["W"])
    if key not in _prog_cache:
        _prog_cache[key] = _build_program(*key)
    nc = _prog_cache[key]
    res = run_bass_kernel_spmd(
        nc, in_maps, core_ids=list(range(N_CORES)), trace=trace
    )
    pooled = _combine(res.results, meta)
    return pooled, res


def kernel(**inputs) -> np.ndarray:
    pooled, _ = _run(inputs, trace=False)
    return pooled
